# revision 4
# baseline (speedup 1.0000x reference)
"""CharLSTM (2-layer, H=256, B=512, T=512) Trainium2 Bass kernel.

Strategy: data-parallel over batch across 8 cores (64 batch/core).
Per core, a software-pipelined wavefront runs layer0 step t and layer1
step t-1 concurrently. All matmuls keep weights stationary (bf16, FWL),
states/gates layout is [4H-on-partitions x batch-on-free], PSUM holds
fp32 gate pre-activations, c-state stays fp32, h-state bf16.

Weight folding done on host:
  - gate rows permuted from (i,f,g,o) to (i,f,o,g) so sigmoid covers a
    contiguous [0:384) free range and tanh [384:512).
  - layer0: embedding lookup E[x] done on host (8-dim, tiny); the
    layer-0 input projection is an extra K=9 matmul chunk [Wih0.T; b0]
    against [emb_t; 1], which also folds the layer-0 bias.
  - layer1 input projection folded into the recurrent matmul as extra
    K-chunks ([Whh1|Wih1] @ [h1; h0]); bias b1 added into PSUM by one
    DVE op per step.
"""

import sys

sys.path.insert(0, "/opt/trn_rl_repo")

from contextlib import ExitStack

import numpy as np
import ml_dtypes

VOCAB = 78
EMBED = 8
H = 256
BATCH = 512
SEQ = 512
NCORES = 8
BPC = BATCH // NCORES  # 64 batch per core
EBLK = 64  # emb prefetch block (steps)

_cache = {}


def _build_program(T):
    import concourse.tile as tile
    import concourse.mybir as mybir
    from concourse import bacc

    dt = mybir.dt
    AF = mybir.ActivationFunctionType
    f32, bf16 = dt.float32, dt.bfloat16

    nc = bacc.Bacc("TRN2", target_bir_lowering=False, debug=False,
                   num_devices=NCORES)

    W0h_d = nc.dram_tensor("W0h", [128, 2, 1024], bf16, kind="ExternalInput").ap()
    W0e_d = nc.dram_tensor("W0e", [9, 1024], bf16, kind="ExternalInput").ap()
    W1_d = nc.dram_tensor("W1", [128, 4, 1024], bf16, kind="ExternalInput").ap()
    B1_d = nc.dram_tensor("B1", [128, 512], f32, kind="ExternalInput").ap()
    Wfc_d = nc.dram_tensor("WfcT", [128, 2, VOCAB], bf16, kind="ExternalInput").ap()
    bfc_d = nc.dram_tensor("bfc", [VOCAB, 1], f32, kind="ExternalInput").ap()
    emb_d = nc.dram_tensor("embT", [9, T * BPC], bf16, kind="ExternalInput").ap()
    out_d = nc.dram_tensor("out", [VOCAB, BPC], f32, kind="ExternalOutput").ap()

    with tile.TileContext(nc) as tc, ExitStack() as ctx:
        const = ctx.enter_context(tc.tile_pool(name="const", bufs=1))
        W0h = const.tile([128, 2, 1024], bf16)
        nc.sync.dma_start(W0h[:], W0h_d)
        W0e = const.tile([9, 1024], bf16)
        nc.sync.dma_start(W0e[:], W0e_d)
        W1 = const.tile([128, 4, 1024], bf16)
        nc.sync.dma_start(W1[:], W1_d)
        B1 = const.tile([128, 512], f32)
        nc.sync.dma_start(B1[:], B1_d)
        WfcT = const.tile([128, 2, VOCAB], bf16)
        nc.sync.dma_start(WfcT[:], Wfc_d)
        bfc = const.tile([VOCAB, 1], f32)
        nc.sync.dma_start(bfc[:], bfc_d)

        embp = ctx.enter_context(tc.tile_pool(name="embp", bufs=2))
        ps0p = ctx.enter_context(tc.tile_pool(name="ps0p", bufs=2, space="PSUM"))
        ps1p = ctx.enter_context(tc.tile_pool(name="ps1p", bufs=2, space="PSUM"))
        psfcp = ctx.enter_context(tc.tile_pool(name="psfcp", bufs=1, space="PSUM"))
        sp = ctx.enter_context(tc.tile_pool(name="sp", bufs=2))
        gp = ctx.enter_context(tc.tile_pool(name="gp", bufs=2))
        cp = ctx.enter_context(tc.tile_pool(name="cp", bufs=2))
        hp = ctx.enter_context(tc.tile_pool(name="hp", bufs=2))
        tp = ctx.enter_context(tc.tile_pool(name="tp", bufs=2))
        tcp = ctx.enter_context(tc.tile_pool(name="tcp", bufs=2))
        fcp = ctx.enter_context(tc.tile_pool(name="fcp", bufs=1))

        eblk = min(EBLK, T)
        nblk = (T + eblk - 1) // eblk
        emb_tiles = [None] * nblk
        h0_prev = c0_prev = h1_prev = c1_prev = None

        for s in range(T + 1):
            # prefetch emb blocks: block 0 at s=0, block b+1 at start of block b
            if s < T and s % eblk == 0:
                b = s // eblk
                if b == 0:
                    e0 = embp.tile([9, eblk * BPC], bf16, name="embblk")
                    nc.sync.dma_start(e0[:], emb_d[:, 0:eblk * BPC])
                    emb_tiles[0] = e0
                if b + 1 < nblk:
                    e1 = embp.tile([9, eblk * BPC], bf16, name="embblk")
                    nc.sync.dma_start(
                        e1[:],
                        emb_d[:, (b + 1) * eblk * BPC:(b + 2) * eblk * BPC])
                    emb_tiles[b + 1] = e1

            h0_in, c0_in = h0_prev, c0_prev  # h0(s-1), c0(s-1)
            h1_in, c1_in = h1_prev, c1_prev  # h1(s-2), c1(s-2)

            if s < T:
                # ---- layer 0, step s ----
                emb_sb = emb_tiles[s // eblk]
                erhs = emb_sb[:, (s % eblk) * BPC:(s % eblk + 1) * BPC]
                ps0 = ps0p.tile([128, 512], f32)
                for m in range(8):
                    o = ps0[:, m * 64:(m + 1) * 64]
                    last = s == 0
                    nc.tensor.matmul(o, W0e[:, m * 128:(m + 1) * 128], erhs,
                                     start=True, stop=last)
                    if s > 0:
                        for k in range(2):
                            nc.tensor.matmul(
                                o, W0h[:, k, m * 128:(m + 1) * 128],
                                h0_in[:, k * 64:(k + 1) * 64],
                                start=False, stop=(k == 1))
                s0 = sp.tile([128, 384], f32, name="s0")
                nc.scalar.activation(s0[:], ps0[:, 0:384], AF.Sigmoid)
                g0 = gp.tile([128, 128], f32, name="g0")
                nc.scalar.activation(g0[:], ps0[:, 384:512], AF.Tanh)
                c0 = cp.tile([128, 128], f32, name="c0")
                if s > 0:
                    t0 = tp.tile([128, 128], f32, name="t0")
                    nc.vector.tensor_mul(t0[:], s0[:, 0:128], g0[:])
                    nc.vector.tensor_mul(c0[:], s0[:, 128:256], c0_in[:])
                    nc.vector.tensor_add(c0[:], c0[:], t0[:])
                else:
                    nc.vector.tensor_mul(c0[:], s0[:, 0:128], g0[:])
                tc0 = tcp.tile([128, 128], f32, name="tc0")
                nc.scalar.activation(tc0[:], c0[:], AF.Tanh)
                h0 = hp.tile([128, 128], bf16, name="h0")
                nc.vector.tensor_mul(h0[:], s0[:, 256:384], tc0[:])
                h0_prev, c0_prev = h0, c0

            if s > 0:
                # ---- layer 1, step s-1 ----
                ps1 = ps1p.tile([128, 512], f32)
                for m in range(8):
                    o = ps1[:, m * 64:(m + 1) * 64]
                    if s > 1:
                        for k in range(2):
                            nc.tensor.matmul(
                                o, W1[:, k, m * 128:(m + 1) * 128],
                                h1_in[:, k * 64:(k + 1) * 64],
                                start=(k == 0), stop=False)
                    for k in range(2):
                        nc.tensor.matmul(
                            o, W1[:, 2 + k, m * 128:(m + 1) * 128],
                            h0_in[:, k * 64:(k + 1) * 64],
                            start=(s == 1 and k == 0), stop=(k == 1))
                nc.vector.tensor_add(ps1[:], ps1[:], B1[:])
                s1 = sp.tile([128, 384], f32, name="s1")
                nc.scalar.activation(s1[:], ps1[:, 0:384], AF.Sigmoid)
                g1 = gp.tile([128, 128], f32, name="g1")
                nc.scalar.activation(g1[:], ps1[:, 384:512], AF.Tanh)
                c1 = cp.tile([128, 128], f32, name="c1")
                if s > 1:
                    t1 = tp.tile([128, 128], f32, name="t1")
                    nc.vector.tensor_mul(t1[:], s1[:, 0:128], g1[:])
                    nc.vector.tensor_mul(c1[:], s1[:, 128:256], c1_in[:])
                    nc.vector.tensor_add(c1[:], c1[:], t1[:])
                else:
                    nc.vector.tensor_mul(c1[:], s1[:, 0:128], g1[:])
                tc1 = tcp.tile([128, 128], f32, name="tc1")
                nc.scalar.activation(tc1[:], c1[:], AF.Tanh)
                h1 = hp.tile([128, 128], bf16, name="h1")
                nc.vector.tensor_mul(h1[:], s1[:, 256:384], tc1[:])
                h1_prev, c1_prev = h1, c1

        # ---- final FC on h1(T-1) ----
        psfc = psfcp.tile([VOCAB, BPC], f32)
        for k in range(2):
            nc.tensor.matmul(psfc[:], WfcT[:, k, :],
                             h1_prev[:, k * 64:(k + 1) * 64],
                             start=(k == 0), stop=(k == 1))
        fc = fcp.tile([VOCAB, BPC], f32)
        nc.scalar.activation(fc[:], psfc[:], AF.Identity, bias=bfc[:])
        nc.sync.dma_start(out_d, fc[:])

    nc.compile()
    return nc


def _prep_inputs(x, E, Wih0, Whh0, bih0, bhh0, Wih1, Whh1, bih1, bhh1,
                 Wfc, bfc, T):
    """Host-side weight folding and per-core input shards."""
    bf16 = ml_dtypes.bfloat16
    # permute gate rows (i,f,g,o) -> (i,f,o,g)
    perm = np.r_[0:256, 256:512, 768:1024, 512:768]
    Wih0 = np.asarray(Wih0, np.float32)[perm]
    Whh0 = np.asarray(Whh0, np.float32)[perm]
    b0 = (np.asarray(bih0, np.float32) + np.asarray(bhh0, np.float32))[perm]
    Wih1 = np.asarray(Wih1, np.float32)[perm]
    Whh1 = np.asarray(Whh1, np.float32)[perm]
    b1 = (np.asarray(bih1, np.float32) + np.asarray(bhh1, np.float32))[perm]
    Wfc = np.asarray(Wfc, np.float32)
    bfc = np.asarray(bfc, np.float32)

    W0h = np.ascontiguousarray(
        Whh0.T.reshape(2, 128, 1024).transpose(1, 0, 2)).astype(bf16)
    W0e = np.concatenate([Wih0.T, b0[None, :]], axis=0).astype(bf16)  # [9,1024]
    W1 = np.ascontiguousarray(
        np.concatenate([Whh1.T, Wih1.T], axis=0)  # [512, 1024]
        .reshape(4, 128, 1024).transpose(1, 0, 2)).astype(bf16)
    B1 = np.ascontiguousarray(
        np.broadcast_to(b1.reshape(8, 128).T[:, :, None],
                        (128, 8, 64)).reshape(128, 512)).astype(np.float32)
    WfcT = np.ascontiguousarray(
        Wfc.T.reshape(2, 128, VOCAB).transpose(1, 0, 2)).astype(bf16)
    bfc2 = np.ascontiguousarray(bfc[:, None]).astype(np.float32)

    E2 = np.asarray(E, np.float32).copy()
    E2[0] = 0.0  # padding_idx=0
    x = np.asarray(x)

    common = {"W0h": W0h, "W0e": W0e, "W1": W1, "B1": B1, "WfcT": WfcT,
              "bfc": bfc2}
    in_maps = []
    for i in range(NCORES):
        xs = x[i * BPC:(i + 1) * BPC, :T]  # [64, T]
        emb = E2[xs]  # [64, T, 8]
        embT = np.empty((9, T, BPC), np.float32)
        embT[:8] = emb.transpose(2, 1, 0)
        embT[8] = 1.0
        m = dict(common)
        m["embT"] = np.ascontiguousarray(embT.reshape(9, T * BPC)).astype(bf16)
        in_maps.append(m)
    return in_maps


def kernel(x, E, Wih0, Whh0, bih0, bhh0, Wih1, Whh1, bih1, bhh1, Wfc, bfc,
           T=SEQ, trace=False):
    from concourse import bass_utils

    if T not in _cache:
        _cache[T] = _build_program(T)
    nc = _cache[T]
    in_maps = _prep_inputs(x, E, Wih0, Whh0, bih0, bhh0, Wih1, Whh1, bih1,
                           bhh1, Wfc, bfc, T)
    res = bass_utils.run_bass_kernel_spmd(nc, in_maps, list(range(NCORES)),
                                          trace=trace)
    out = np.empty((BATCH, VOCAB), np.float32)
    for i in range(NCORES):
        out[i * BPC:(i + 1) * BPC] = np.asarray(res.results[i]["out"]).T
    if trace:
        return out, res
    return out


# revision 11
# speedup vs baseline: 1.6227x; 1.6227x over previous
"""CharLSTM (2-layer, H=256, B=512, T=512) Trainium2 Bass kernel.

Strategy: data-parallel over batch across 8 cores (64 batch/core).
Per core, a software-pipelined wavefront runs layer0 step t and layer1
step t-1 concurrently. All matmuls keep weights stationary (bf16, FWL),
states/gates layout is [4H-on-partitions x batch-on-free], PSUM holds
fp32 gate pre-activations, c-state stays fp32, h-state bf16.

Weight folding done on host:
  - gate rows permuted from (i,f,g,o) to (i,f,o,g) so sigmoid covers a
    contiguous [0:384) free range and tanh [384:512).
  - layer0: embedding lookup E[x] done on host (8-dim, tiny); the
    layer-0 input projection is an extra K=9 matmul chunk [Wih0.T; b0]
    against [emb_t; 1], which also folds the layer-0 bias.
  - layer1 input projection folded into the recurrent matmul as extra
    K-chunks ([Whh1|Wih1] @ [h1; h0]); bias b1 added into PSUM by one
    DVE op per step.
"""

import sys

sys.path.insert(0, "/opt/trn_rl_repo")

from contextlib import ExitStack

import numpy as np
import ml_dtypes

VOCAB = 78
EMBED = 8
H = 256
BATCH = 512
SEQ = 512
NCORES = 8
BPC = BATCH // NCORES  # 64 batch per core
EBLK = 64  # emb prefetch block (steps)

_cache = {}


def _build_program(T):
    import concourse.tile as tile
    import concourse.mybir as mybir
    from concourse import bacc

    dt = mybir.dt
    AF = mybir.ActivationFunctionType
    f32, bf16 = dt.float32, dt.bfloat16

    nc = bacc.Bacc("TRN2", target_bir_lowering=False, debug=False,
                   num_devices=NCORES)

    W0h_d = nc.dram_tensor("W0h", [128, 2, 1024], bf16, kind="ExternalInput").ap()
    W0e_d = nc.dram_tensor("W0e", [9, 1024], bf16, kind="ExternalInput").ap()
    W1_d = nc.dram_tensor("W1", [128, 4, 1024], bf16, kind="ExternalInput").ap()
    B1_d = nc.dram_tensor("B1", [128, 512], f32, kind="ExternalInput").ap()
    Wfc_d = nc.dram_tensor("WfcT", [128, 2, VOCAB], bf16, kind="ExternalInput").ap()
    bfc_d = nc.dram_tensor("bfc", [VOCAB, 1], f32, kind="ExternalInput").ap()
    emb_d = nc.dram_tensor("embT", [9, T * BPC], bf16, kind="ExternalInput").ap()
    out_d = nc.dram_tensor("out", [VOCAB, BPC], f32, kind="ExternalOutput").ap()

    with tile.TileContext(nc) as tc, ExitStack() as ctx:
        const = ctx.enter_context(tc.tile_pool(name="const", bufs=1))
        W0h = const.tile([128, 2, 1024], bf16)
        nc.sync.dma_start(W0h[:], W0h_d)
        W0e = const.tile([9, 1024], bf16)
        nc.sync.dma_start(W0e[:], W0e_d)
        W1 = const.tile([128, 4, 1024], bf16)
        nc.sync.dma_start(W1[:], W1_d)
        B1 = const.tile([128, 512], f32)
        nc.sync.dma_start(B1[:], B1_d)
        WfcT = const.tile([128, 2, VOCAB], bf16)
        nc.sync.dma_start(WfcT[:], Wfc_d)
        bfc = const.tile([VOCAB, 1], f32)
        nc.sync.dma_start(bfc[:], bfc_d)

        embp = ctx.enter_context(tc.tile_pool(name="embp", bufs=2))
        ps0p = ctx.enter_context(tc.tile_pool(name="ps0p", bufs=3, space="PSUM"))
        ps1p = ctx.enter_context(tc.tile_pool(name="ps1p", bufs=2, space="PSUM"))
        psfcp = ctx.enter_context(tc.tile_pool(name="psfcp", bufs=1, space="PSUM"))
        sp = ctx.enter_context(tc.tile_pool(name="sp", bufs=2))
        gp = ctx.enter_context(tc.tile_pool(name="gp", bufs=2))
        cp = ctx.enter_context(tc.tile_pool(name="cp", bufs=2))
        hp = ctx.enter_context(tc.tile_pool(name="hp", bufs=2))
        tp = ctx.enter_context(tc.tile_pool(name="tp", bufs=2))
        tcp = ctx.enter_context(tc.tile_pool(name="tcp", bufs=2))
        fcp = ctx.enter_context(tc.tile_pool(name="fcp", bufs=1))

        eblk = min(EBLK, T)
        nblk = (T + eblk - 1) // eblk
        emb_tiles = [None] * nblk
        ps0_tiles = {}
        h0_prev = c0_prev = h1_prev = c1_prev = None

        def emit_emb(step):
            # layer-0 input-projection matmuls for `step`, into a fresh ps0
            # tile. One start=True per PSUM bank: start clears has_written
            # for the whole bank, so only the first MM into the bank may set
            # it; per-element has_written then handles overwrite-vs-accum
            # for every later MM (emb slices and the step-`step` h-matmuls).
            ps0 = ps0p.tile([128, 512], f32, name="ps0")
            ps0_tiles[step] = ps0
            emb_sb = emb_tiles[step // eblk]
            erhs = emb_sb[:, (step % eblk) * BPC:(step % eblk + 1) * BPC]
            for m in range(8):
                nc.tensor.matmul(ps0[:, m * 64:(m + 1) * 64],
                                 W0e[:, m * 128:(m + 1) * 128], erhs,
                                 start=(m == 0),
                                 stop=(step == 0 and m == 7),
                                 skip_group_check=True)

        for s in range(T + 1):
            # prefetch emb blocks: block 0 at s=0, block b+1 at start of block b
            if s < T and s % eblk == 0:
                b = s // eblk
                if b == 0:
                    e0 = embp.tile([9, eblk * BPC], bf16, name="embblk")
                    nc.sync.dma_start(e0[:], emb_d[:, 0:eblk * BPC])
                    emb_tiles[0] = e0
                if b + 1 < nblk:
                    e1 = embp.tile([9, eblk * BPC], bf16, name="embblk")
                    nc.sync.dma_start(
                        e1[:],
                        emb_d[:, (b + 1) * eblk * BPC:(b + 2) * eblk * BPC])
                    emb_tiles[b + 1] = e1

            h0_in, c0_in = h0_prev, c0_prev  # h0(s-1), c0(s-1)
            h1_in, c1_in = h1_prev, c1_prev  # h1(s-2), c1(s-2)

            if s == 0:
                emit_emb(0)
                if T > 1:
                    emit_emb(1)

            if s < T:
                # ---- layer 0, step s: recurrent matmuls ----
                ps0 = ps0_tiles.pop(s)
                if s > 0:
                    for m in range(8):
                        o = ps0[:, m * 64:(m + 1) * 64]
                        for k in range(2):
                            nc.tensor.matmul(
                                o, W0h[:, k, m * 128:(m + 1) * 128],
                                h0_in[:, k * 64:(k + 1) * 64],
                                start=False, stop=(m == 7 and k == 1),
                                skip_group_check=True)
                s0 = sp.tile([128, 384], bf16, name="s0")
                nc.scalar.activation(s0[:], ps0[:, 0:384], AF.Sigmoid)
                g0 = gp.tile([128, 128], bf16, name="g0")
                nc.scalar.activation(g0[:], ps0[:, 384:512], AF.Tanh)
                c0 = cp.tile([128, 128], f32, name="c0")
                if s > 0:
                    t0 = tp.tile([128, 128], bf16, name="t0")
                    nc.vector.tensor_mul(c0[:], s0[:, 128:256], c0_in[:])
                    nc.vector.tensor_mul(t0[:], s0[:, 0:128], g0[:])
                    nc.vector.tensor_add(c0[:], c0[:], t0[:])
                else:
                    nc.vector.tensor_mul(c0[:], s0[:, 0:128], g0[:])
                tc0 = tcp.tile([128, 128], bf16, name="tc0")
                nc.scalar.activation(tc0[:], c0[:], AF.Tanh)
                h0 = hp.tile([128, 128], bf16, name="h0")
                nc.vector.tensor_mul(h0[:], s0[:, 256:384], tc0[:])
                h0_prev, c0_prev = h0, c0

            if s > 0:
                # ---- layer 1, step s-1 ----
                ps1 = ps1p.tile([128, 512], f32)
                for m in range(8):
                    o = ps1[:, m * 64:(m + 1) * 64]
                    if s > 1:
                        for k in range(2):
                            nc.tensor.matmul(
                                o, W1[:, k, m * 128:(m + 1) * 128],
                                h1_in[:, k * 64:(k + 1) * 64],
                                start=(k == 0), stop=False)
                    for k in range(2):
                        nc.tensor.matmul(
                            o, W1[:, 2 + k, m * 128:(m + 1) * 128],
                            h0_in[:, k * 64:(k + 1) * 64],
                            start=(s == 1 and k == 0), stop=(k == 1))
                nc.vector.tensor_add(ps1[:], ps1[:], B1[:])
                s1 = sp.tile([128, 384], bf16, name="s1")
                nc.scalar.activation(s1[:], ps1[:, 0:384], AF.Sigmoid)
                g1 = gp.tile([128, 128], bf16, name="g1")
                nc.scalar.activation(g1[:], ps1[:, 384:512], AF.Tanh)
                c1 = cp.tile([128, 128], f32, name="c1")
                if s > 1:
                    t1 = tp.tile([128, 128], bf16, name="t1")
                    nc.vector.tensor_mul(c1[:], s1[:, 128:256], c1_in[:])
                    nc.vector.tensor_mul(t1[:], s1[:, 0:128], g1[:])
                    nc.vector.tensor_add(c1[:], c1[:], t1[:])
                else:
                    nc.vector.tensor_mul(c1[:], s1[:, 0:128], g1[:])
                tc1 = tcp.tile([128, 128], bf16, name="tc1")
                nc.scalar.activation(tc1[:], c1[:], AF.Tanh)
                h1 = hp.tile([128, 128], bf16, name="h1")
                nc.vector.tensor_mul(h1[:], s1[:, 256:384], tc1[:])
                h1_prev, c1_prev = h1, c1

            if s + 2 < T:
                emit_emb(s + 2)


        # ---- final FC on h1(T-1) ----
        psfc = psfcp.tile([VOCAB, BPC], f32)
        for k in range(2):
            nc.tensor.matmul(psfc[:], WfcT[:, k, :],
                             h1_prev[:, k * 64:(k + 1) * 64],
                             start=(k == 0), stop=(k == 1))
        fc = fcp.tile([VOCAB, BPC], f32)
        nc.scalar.activation(fc[:], psfc[:], AF.Identity, bias=bfc[:])
        nc.sync.dma_start(out_d, fc[:])

    nc.compile()
    return nc


def _prep_inputs(x, E, Wih0, Whh0, bih0, bhh0, Wih1, Whh1, bih1, bhh1,
                 Wfc, bfc, T):
    """Host-side weight folding and per-core input shards."""
    bf16 = ml_dtypes.bfloat16
    # permute gate rows (i,f,g,o) -> (i,f,o,g)
    perm = np.r_[0:256, 256:512, 768:1024, 512:768]
    Wih0 = np.asarray(Wih0, np.float32)[perm]
    Whh0 = np.asarray(Whh0, np.float32)[perm]
    b0 = (np.asarray(bih0, np.float32) + np.asarray(bhh0, np.float32))[perm]
    Wih1 = np.asarray(Wih1, np.float32)[perm]
    Whh1 = np.asarray(Whh1, np.float32)[perm]
    b1 = (np.asarray(bih1, np.float32) + np.asarray(bhh1, np.float32))[perm]
    Wfc = np.asarray(Wfc, np.float32)
    bfc = np.asarray(bfc, np.float32)

    W0h = np.ascontiguousarray(
        Whh0.T.reshape(2, 128, 1024).transpose(1, 0, 2)).astype(bf16)
    W0e = np.concatenate([Wih0.T, b0[None, :]], axis=0).astype(bf16)  # [9,1024]
    W1 = np.ascontiguousarray(
        np.concatenate([Whh1.T, Wih1.T], axis=0)  # [512, 1024]
        .reshape(4, 128, 1024).transpose(1, 0, 2)).astype(bf16)
    B1 = np.ascontiguousarray(
        np.broadcast_to(b1.reshape(8, 128).T[:, :, None],
                        (128, 8, 64)).reshape(128, 512)).astype(np.float32)
    WfcT = np.ascontiguousarray(
        Wfc.T.reshape(2, 128, VOCAB).transpose(1, 0, 2)).astype(bf16)
    bfc2 = np.ascontiguousarray(bfc[:, None]).astype(np.float32)

    E2 = np.asarray(E, np.float32).copy()
    E2[0] = 0.0  # padding_idx=0
    x = np.asarray(x)

    common = {"W0h": W0h, "W0e": W0e, "W1": W1, "B1": B1, "WfcT": WfcT,
              "bfc": bfc2}
    in_maps = []
    for i in range(NCORES):
        xs = x[i * BPC:(i + 1) * BPC, :T]  # [64, T]
        emb = E2[xs]  # [64, T, 8]
        embT = np.empty((9, T, BPC), np.float32)
        embT[:8] = emb.transpose(2, 1, 0)
        embT[8] = 1.0
        m = dict(common)
        m["embT"] = np.ascontiguousarray(embT.reshape(9, T * BPC)).astype(bf16)
        in_maps.append(m)
    return in_maps


def kernel(x, E, Wih0, Whh0, bih0, bhh0, Wih1, Whh1, bih1, bhh1, Wfc, bfc,
           T=SEQ, trace=False):
    from concourse import bass_utils

    if T not in _cache:
        _cache[T] = _build_program(T)
    nc = _cache[T]
    in_maps = _prep_inputs(x, E, Wih0, Whh0, bih0, bhh0, Wih1, Whh1, bih1,
                           bhh1, Wfc, bfc, T)
    res = bass_utils.run_bass_kernel_spmd(nc, in_maps, list(range(NCORES)),
                                          trace=trace)
    out = np.empty((BATCH, VOCAB), np.float32)
    for i in range(NCORES):
        out[i * BPC:(i + 1) * BPC] = np.asarray(res.results[i]["out"]).T
    if trace:
        return out, res
    return out
